# revision 21
# baseline (speedup 1.0000x reference)
"""Nearest-neighbor tokenizer on 8 Trainium2 NeuronCores.

Math: d2[t,m] = ||x_t||^2 + ||c_m||^2 - 2 x_t.c_m over 65536 tokens x 4096 codes.
out[t] = argmin_m d2 if min d2 <= 0.1 else -1.

Fast path (per 128-token block, 8 PSUM banks of 512 codes each):
  g[t,m] = x_t.c_m - ||c_m||^2/2 via one GEMM with a ones row on x and
  a -c2/2 row on codes^T, host-transposed to bf16 and zero-padded to
  K=128 contraction rows (K<96 leaves the PE clock throttled at 1.2GHz).
  Every consumer computes the same certificate sum(relu(g - tau_t)) with
  tau_t = (||x_t||^2 - 0.1)/2 - margin: exactly 0 iff no code is within
  threshold. Banks 0-3 -> DVE tensor_scalar(add tau, max 0)+accum_out;
  banks 4-5 -> ACT relu strip folded by GPSIMD tensor_tensor adds;
  banks 6-7 -> ACT relu+accum_out. Four engines run ~balanced at
  ~2.5us/block. Host checks all partials ~= 0; any potential match
  falls back to the full argmax program (device argmax + exact fp64
  threshold recheck on host).

Sharding: data-parallel over tokens. Core c gets batches [2c, 2c+2) ->
a contiguous slab of 8192 tokens; the codebook is replicated.
"""

import os

import numpy as np

B, N, D = 16, 4096, 64
M = 4096
NCORES = 8
TOK = B * N // NCORES          # 8192 tokens per core
NBLK = TOK // 128              # 64 blocks of 128 tokens
NCH = M // 512                 # 8 chunks of 512 codes
CBLK = M // 128                # 32 code blocks
THRESH = 0.1
FALLBACK_MARGIN = 2.0
SOFT = 0.5                     # bf16 score-error safety margin

_CACHE = {}


def _build(stage=6):
    import concourse.bacc as bacc
    import concourse.mybir as mybir
    import concourse.tile as tile
    from contextlib import ExitStack

    fp32 = mybir.dt.float32
    bf16 = mybir.dt.bfloat16
    i32 = mybir.dt.int32
    u32 = mybir.dt.uint32
    Alu = mybir.AluOpType
    Act = mybir.ActivationFunctionType

    nc = bacc.Bacc(
        "TRN2",
        target_bir_lowering=False,
        debug=False,
        enable_asserts=False,
        num_devices=1,
    )

    x_d = nc.dram_tensor("x", (TOK, D), fp32, kind="ExternalInput")
    c_d = nc.dram_tensor("codes", (M, D), fp32, kind="ExternalInput")
    id_d = nc.dram_tensor("ident", (128, 128), fp32, kind="ExternalInput")
    o_d = nc.dram_tensor("out", (TOK,), u32, kind="ExternalOutput")

    with tile.TileContext(nc) as tc, ExitStack() as ctx:
        sb = ctx.enter_context(tc.tile_pool(name="sb", bufs=1))

        ident = sb.tile((128, 128), fp32, tag="ident")
        xsb = sb.tile((128, NBLK, D), fp32, tag="xsb")
        csb = sb.tile((128, CBLK, D), fp32, tag="csb")
        xT = sb.tile((65, NBLK * 128), bf16, tag="xT")
        cT = sb.tile((65, M), bf16, tag="cT")
        cTsq = sb.tile((64, M), bf16, tag="cTsq")
        ones64 = sb.tile((64, 1), bf16, tag="ones64")
        x2 = sb.tile((128, NBLK), fp32, tag="x2")
        sq_all = sb.tile((128, NBLK, D), fp32, tag="sq_all")
        out_sb = sb.tile((128, NBLK), u32, tag="out_sb")
        top8 = sb.tile((128, 8), bf16, tag="top8")
        idx8 = sb.tile((128, 8), u32, tag="idx8")
        gmaxf = sb.tile((128, 1), fp32, tag="gmaxf")
        mind2 = sb.tile((128, 1), fp32, tag="mind2")
        mask = sb.tile((128, 1), mybir.dt.uint8, tag="mask")

        dma = nc.default_dma_engine
        dma.dma_start(out=ident, in_=id_d[:, :])
        dma.dma_start(out=xsb, in_=x_d[:, :].rearrange("(b p) d -> p b d", p=128))
        dma.dma_start(out=csb, in_=c_d[:, :].rearrange("(b p) d -> p b d", p=128))

        nc.vector.memset(xT[64:65, :], 1.0)
        nc.vector.memset(ones64, 1.0)
        nc.vector.memset(out_sb, 0xFFFFFFFF)

        # --- setup: transpose codes and x into [d, token/code] bf16 layout ---
        if stage >= 2:
            with tc.tile_pool(name="tpsum", bufs=4, space="PSUM") as tp:
                for cb in range(CBLK):
                    pt = tp.tile((64, 128), fp32, tag="ct")
                    nc.tensor.transpose(pt, csb[:, cb, :], ident)
                    nc.scalar.copy(cT[0:64, cb * 128:(cb + 1) * 128], pt)
                for xb in range(NBLK):
                    pt = tp.tile((64, 128), fp32, tag="xt")
                    nc.tensor.transpose(pt, xsb[:, xb, :], ident)
                    nc.scalar.copy(xT[0:64, xb * 128:(xb + 1) * 128], pt)

            # cTsq = cT*cT, c2 row: ones.T @ cTsq -> -c2/2 into cT row 64
            nc.vector.tensor_tensor(cTsq, cT[0:64, :], cT[0:64, :], op=Alu.mult)
            with tc.tile_pool(name="c2psum", bufs=2, space="PSUM") as cp:
                for j in range(NCH):
                    pt = cp.tile((1, 512), fp32, tag="c2")
                    nc.tensor.matmul(pt, ones64, cTsq[:, j * 512:(j + 1) * 512],
                                     start=True, stop=True)
                    nc.scalar.activation(cT[64:65, j * 512:(j + 1) * 512], pt,
                                         Act.Copy, bias=0.0, scale=-0.5)

        # x2[t] = sum_d x^2 (fp32): ACT square whole slab, DVE reduce innermost
        if stage >= 3:
            nc.scalar.activation(sq_all, xsb, Act.Square, bias=0.0, scale=1.0)
            nc.vector.tensor_reduce(x2, sq_all, axis=mybir.AxisListType.X,
                                    op=Alu.add)
        else:
            nc.vector.memset(x2, 1.0)

        # --- main loop ---
        if stage >= 4:
            with tc.tile_pool(name="gpsum", bufs=1, space="PSUM") as gp, \
                 tc.tile_pool(name="gsb", bufs=2) as gsb_pool:
                gbanks = [gp.tile((128, 512), fp32, tag=f"g{j}", name=f"g{j}")
                          for j in range(NCH)]
                for blk in range(NBLK):
                    lhsT = xT[:, blk * 128:(blk + 1) * 128]
                    g_sb = gsb_pool.tile((128, M), bf16, tag="g_sb")
                    for j in range(NCH):
                        nc.tensor.matmul(gbanks[j], lhsT,
                                         cT[:, j * 512:(j + 1) * 512],
                                         start=True, stop=True)
                        nc.scalar.copy(g_sb[:, j * 512:(j + 1) * 512], gbanks[j])
                    if stage >= 5:
                        nc.vector.max(top8, g_sb)
                        nc.vector.max_index(idx8, top8, g_sb)
                    if stage >= 6:
                        # argmax id only; the threshold decision happens on
                        # the host with an exact fp64 distance recompute
                        # (bf16 scores are too noisy for borderline cases)
                        nc.vector.tensor_copy(out_sb[:, blk:blk + 1],
                                              idx8[:, 0:1])

        dma.dma_start(out=o_d[:].rearrange("(b p) -> p b", p=128), in_=out_sb)

    nc.compile()
    return nc


KPAD = 128  # contraction rows incl. zero padding: K<96 leaves the PE
            # HAM activity monitor throttled at 1.2 GHz (measured), K>=96
            # runs warm at 2.4 GHz.
VW = 976    # per-instruction span for the two VectorE consumers
SW = (4096 - 2 * VW) // 2  # per-instruction span for the two ScalarE ones


def _build_fast2():
    """Screening program: no argmax, no on-device setup.

    Inputs are host-transposed bf16 padded to KPAD contraction rows:
    xt (KPAD, TOK) row 64 = ones, ct (KPAD, M) row 64 = -c2/2, rows
    65+ zero; tau (128, NBLK) = -((x2-0.1)/2 - SOFT).
    Per block: 8 matmuls -> 4 flat 2-bank PSUM tiles; DVE flat
    tensor_reduce(max) tiles 0-1 -> gmax partials (codes 0-2047); ACT
    relu+accum tiles 2-3 -> sum-of-relu partials (codes 2048-4095).
    """
    import concourse.bacc as bacc
    import concourse.mybir as mybir
    import concourse.tile as tile
    from contextlib import ExitStack

    fp32 = mybir.dt.float32
    bf16 = mybir.dt.bfloat16
    Alu = mybir.AluOpType
    Act = mybir.ActivationFunctionType

    nc = bacc.Bacc(
        "TRN2",
        target_bir_lowering=False,
        debug=False,
        enable_asserts=False,
        num_devices=1,
    )

    xt_d = nc.dram_tensor("xt", (KPAD, TOK), bf16, kind="ExternalInput")
    ct_d = nc.dram_tensor("ct", (KPAD, M), bf16, kind="ExternalInput")
    tau_d = nc.dram_tensor("tau", (128, NBLK), fp32, kind="ExternalInput")
    outp_d = nc.dram_tensor("outp", (128, NBLK, 4), fp32, kind="ExternalOutput")

    with tile.TileContext(nc) as tc, ExitStack() as ctx:
        sb = ctx.enter_context(tc.tile_pool(name="sb", bufs=1))

        xt_sb = sb.tile((KPAD, TOK), bf16, tag="xt_sb")
        ct_sb = sb.tile((KPAD, M), bf16, tag="ct_sb")
        tau_sb = sb.tile((128, NBLK), fp32, tag="tau_sb")
        outp_sb = sb.tile((128, NBLK, 4), fp32, tag="outp_sb")
        gacc = sb.tile((128, SW), bf16, tag="gacc")
        warm = sb.tile((128, 1), fp32, tag="warm")
        dmy = sb.tile((128, 512), bf16, tag="dmy")

        dma = nc.default_dma_engine
        # ACT table warm-up: pay the one-time table load under the DMA wait;
        # GPSIMD warm-up: trigger its ucode load early; zero accumulators.
        nc.vector.memset(warm, 0.0)
        nc.scalar.activation(warm, warm, Act.Relu, bias=0.0, scale=1.0)
        nc.vector.memset(outp_sb, 0.0)
        nc.vector.memset(dmy, 0.0)
        nc.gpsimd.memset(gacc, 0.0)
        nc.gpsimd.tensor_tensor(gacc[:, 0:8], gacc[:, 0:8], gacc[:, 0:8],
                                op=Alu.add)

        # DMA order: first block's operands first, finely chunked so early
        # matmuls are not gated on one large transfer; then the rest in a
        # few large transfers (each dma_start costs ~0.7us of issue time).
        dma.dma_start(out=ct_sb[:, 0:512], in_=ct_d[:, 0:512])
        dma.dma_start(out=xt_sb[:, 0:128], in_=xt_d[:, 0:128])
        dma.dma_start(out=tau_sb, in_=tau_d[:, :])
        dma.dma_start(out=ct_sb[:, 512:1024], in_=ct_d[:, 512:1024])
        dma.dma_start(out=ct_sb[:, 1024:2048], in_=ct_d[:, 1024:2048])
        dma.dma_start(out=xt_sb[:, 128:1024], in_=xt_d[:, 128:1024])
        dma.dma_start(out=ct_sb[:, 2048:4096], in_=ct_d[:, 2048:4096])
        dma.dma_start(out=xt_sb[:, 1024:4096], in_=xt_d[:, 1024:4096])
        dma.dma_start(out=xt_sb[:, 4096:TOK], in_=xt_d[:, 4096:TOK])

        with tc.tile_pool(name="gpsum", bufs=1, space="PSUM") as gp, \
             tc.tile_pool(name="scr", bufs=2) as scr_pool:
            pall = gp.tile((128, 4096), fp32, tag="pall", name="pall")
            # dummy matmuls during the DMA wait: build up the PE activity
            # window so the HAM clock un-throttles before real work arrives
            for w in range(5):
                nc.tensor.matmul(pall[:, 3584:4096], dmy[:, 0:128], dmy,
                                 start=True, stop=True)
            for blk in range(NBLK):
                lhsT = xt_sb[:, blk * 128:(blk + 1) * 128]
                for j in range(NCH):
                    nc.tensor.matmul(pall[:, j * 512:(j + 1) * 512], lhsT,
                                     ct_sb[:, j * 512:(j + 1) * 512],
                                     start=True, stop=True)
                # DVE: relu(g - tau) + accum over the first ~3.8 banks;
                # ScalarE (faster per element) takes the remainder
                for h in range(2):
                    lo, hi = h * VW, (h + 1) * VW
                    vscr = scr_pool.tile((128, VW), bf16, tag=f"v{h}")
                    nc.vector.tensor_scalar(out=vscr, in0=pall[:, lo:hi],
                                            scalar1=tau_sb[:, blk:blk + 1],
                                            scalar2=0.0,
                                            op0=Alu.add, op1=Alu.max,
                                            accum_out=outp_sb[:, blk,
                                                              h:h + 1])
                # ACT: relu(g - tau) over banks 4-5, strip folded by GPSIMD
                # (saves one accumulator-read per block on ScalarE)
                # the last block's strip accumulates directly so the gacc
                # fold (strips 0-62) runs one block early, off the tail
                last = blk == NBLK - 1
                if last:
                    gscr = scr_pool.tile((128, SW), bf16, tag="gscr")
                    nc.vector.tensor_scalar(out=gscr, in0=gacc,
                                            scalar1=0.0, scalar2=0.0,
                                            op0=Alu.add, op1=Alu.max,
                                            accum_out=outp_sb[:, blk - 1,
                                                              3:4])
                s1 = scr_pool.tile((128, SW), bf16, tag="s1")
                nc.scalar.activation(s1, pall[:, 2 * VW:2 * VW + SW], Act.Relu,
                                     bias=tau_sb[:, blk:blk + 1], scale=1.0,
                                     accum_out=(outp_sb[:, blk, 3:4]
                                                if last else None))
                if not last:
                    nc.gpsimd.tensor_tensor(gacc, gacc, s1, op=Alu.add)
                # ACT: relu(g - tau) + accum over banks 6-7
                s2 = scr_pool.tile((128, SW), bf16, tag="s2")
                nc.scalar.activation(s2, pall[:, 2 * VW + SW:4096], Act.Relu,
                                     bias=tau_sb[:, blk:blk + 1], scale=1.0,
                                     accum_out=outp_sb[:, blk, 2:3])

        dma.dma_start(out=outp_d[:, :, :], in_=outp_sb)

    nc.compile()
    return nc


def _run(nc, in_maps, trace):
    from concourse import bass_utils
    try:
        return bass_utils.run_bass_kernel_spmd(
            nc, in_maps, list(range(NCORES)), trace=trace)
    except Exception:
        if not trace:
            raise
        return bass_utils.run_bass_kernel_spmd(
            nc, in_maps, list(range(NCORES)), trace=False)


def _full_in_maps(x, codes):
    ident = np.eye(128, dtype=np.float32)
    xf = x.reshape(NCORES, TOK, D)
    return [
        {"x": xf[c], "codes": codes, "ident": ident}
        for c in range(NCORES)
    ]


def _run_full(x, codes, trace):
    if "full" not in _CACHE:
        _CACHE["full"] = _build(6)
    res = _run(_CACHE["full"], _full_in_maps(x, codes), trace)
    _CACHE["last_res"] = res
    ids = np.concatenate(
        [np.asarray(res.results[c]["out"], dtype=np.uint32)
         for c in range(NCORES)]).astype(np.int64)
    # exact threshold decision on the device-picked nearest candidates
    xf = x.reshape(-1, D).astype(np.float64)
    d2 = ((xf - codes.astype(np.float64)[ids]) ** 2).sum(axis=1)
    out = np.where(d2 <= THRESH, ids, -1).astype(np.int32)
    return out.reshape(B, N)


def kernel(x: np.ndarray, codes: np.ndarray) -> np.ndarray:
    import ml_dtypes

    os.environ.setdefault("NEURON_RT_RESET_CORES", "1")
    x = np.ascontiguousarray(x, dtype=np.float32)
    codes = np.ascontiguousarray(codes, dtype=np.float32)
    trace = bool(os.environ.get("KERNEL_TRACE"))

    if os.environ.get("KERNEL_FORCE_FULL"):
        return _run_full(x, codes, trace)

    bf16 = ml_dtypes.bfloat16
    xf = x.reshape(NCORES, TOK, D)

    ct = np.zeros((KPAD, M), dtype=bf16)
    ct[0:64] = codes.T.astype(bf16)
    c2 = (codes.astype(np.float64) ** 2).sum(axis=1)
    ct[64] = (-0.5 * c2).astype(bf16)

    in_maps = []
    for c in range(NCORES):
        slab = xf[c]
        xt = np.zeros((KPAD, TOK), dtype=bf16)
        xt[0:64] = slab.T.astype(bf16)
        xt[64] = np.float32(1.0)
        x2 = (slab.astype(np.float64) ** 2).sum(axis=1)          # (TOK,)
        x2_pb = x2.reshape(NBLK, 128).T.astype(np.float32)       # (128, NBLK)
        tau = np.ascontiguousarray(
            -((x2_pb - THRESH) * 0.5 - SOFT).astype(np.float32))
        in_maps.append({"xt": xt, "ct": np.ascontiguousarray(ct), "tau": tau})

    if "fast2" not in _CACHE:
        _CACHE["fast2"] = _build_fast2()
    res = _run(_CACHE["fast2"], in_maps, trace)
    _CACHE["last_res"] = res

    ok = True
    for c in range(NCORES):
        outp = np.asarray(res.results[c]["outp"], dtype=np.float32)
        # sum-of-relu certificate: exactly 0 iff every code is far from
        # every token (any true match contributes >= SOFT - bf16 error)
        if outp.max() > 0.05:
            ok = False
            break
    if ok:
        return np.full((B, N), -1, dtype=np.int32)

    return _run_full(x, codes, trace)


# revision 22
# speedup vs baseline: 1.2920x; 1.2920x over previous
"""Nearest-neighbor tokenizer on 8 Trainium2 NeuronCores.

Math: d2[t,m] = ||x_t||^2 + ||c_m||^2 - 2 x_t.c_m over 65536 tokens x 4096 codes.
out[t] = argmin_m d2 if min d2 <= 0.1 else -1.

Fast path (per 128-token block, 8 PSUM banks of 512 codes each):
  g[t,m] = x_t.c_m - ||c_m||^2/2 via one GEMM with a ones row on x and
  a -c2/2 row on codes^T, host-transposed to bf16 and zero-padded to
  K=128 contraction rows (K<96 leaves the PE clock throttled at 1.2GHz).
  Every consumer computes the same certificate sum(relu(g - tau_t)) with
  tau_t = (||x_t||^2 - 0.1)/2 - margin: exactly 0 iff no code is within
  threshold. Banks 0-3 -> DVE tensor_scalar(add tau, max 0)+accum_out;
  banks 4-5 -> ACT relu strip folded by GPSIMD tensor_tensor adds;
  banks 6-7 -> ACT relu+accum_out. Four engines run ~balanced at
  ~2.5us/block. Host checks all partials ~= 0; any potential match
  falls back to the full argmax program (device argmax + exact fp64
  threshold recheck on host).

Sharding: data-parallel over tokens. Core c gets batches [2c, 2c+2) ->
a contiguous slab of 8192 tokens; the codebook is replicated.
"""

import os

import numpy as np

B, N, D = 16, 4096, 64
M = 4096
NCORES = 8
TOK = B * N // NCORES          # 8192 tokens per core
NBLK = TOK // 128              # 64 blocks of 128 tokens
NCH = M // 512                 # 8 chunks of 512 codes
CBLK = M // 128                # 32 code blocks
THRESH = 0.1
FALLBACK_MARGIN = 2.0
SOFT = 0.5                     # bf16 score-error safety margin

_CACHE = {}


def _build(stage=6):
    import concourse.bacc as bacc
    import concourse.mybir as mybir
    import concourse.tile as tile
    from contextlib import ExitStack

    fp32 = mybir.dt.float32
    bf16 = mybir.dt.bfloat16
    i32 = mybir.dt.int32
    u32 = mybir.dt.uint32
    Alu = mybir.AluOpType
    Act = mybir.ActivationFunctionType

    nc = bacc.Bacc(
        "TRN2",
        target_bir_lowering=False,
        debug=False,
        enable_asserts=False,
        num_devices=1,
    )

    x_d = nc.dram_tensor("x", (TOK, D), fp32, kind="ExternalInput")
    c_d = nc.dram_tensor("codes", (M, D), fp32, kind="ExternalInput")
    id_d = nc.dram_tensor("ident", (128, 128), fp32, kind="ExternalInput")
    o_d = nc.dram_tensor("out", (TOK,), u32, kind="ExternalOutput")

    with tile.TileContext(nc) as tc, ExitStack() as ctx:
        sb = ctx.enter_context(tc.tile_pool(name="sb", bufs=1))

        ident = sb.tile((128, 128), fp32, tag="ident")
        xsb = sb.tile((128, NBLK, D), fp32, tag="xsb")
        csb = sb.tile((128, CBLK, D), fp32, tag="csb")
        xT = sb.tile((65, NBLK * 128), bf16, tag="xT")
        cT = sb.tile((65, M), bf16, tag="cT")
        cTsq = sb.tile((64, M), bf16, tag="cTsq")
        ones64 = sb.tile((64, 1), bf16, tag="ones64")
        x2 = sb.tile((128, NBLK), fp32, tag="x2")
        sq_all = sb.tile((128, NBLK, D), fp32, tag="sq_all")
        out_sb = sb.tile((128, NBLK), u32, tag="out_sb")
        top8 = sb.tile((128, 8), bf16, tag="top8")
        idx8 = sb.tile((128, 8), u32, tag="idx8")
        gmaxf = sb.tile((128, 1), fp32, tag="gmaxf")
        mind2 = sb.tile((128, 1), fp32, tag="mind2")
        mask = sb.tile((128, 1), mybir.dt.uint8, tag="mask")

        dma = nc.default_dma_engine
        dma.dma_start(out=ident, in_=id_d[:, :])
        dma.dma_start(out=xsb, in_=x_d[:, :].rearrange("(b p) d -> p b d", p=128))
        dma.dma_start(out=csb, in_=c_d[:, :].rearrange("(b p) d -> p b d", p=128))

        nc.vector.memset(xT[64:65, :], 1.0)
        nc.vector.memset(ones64, 1.0)
        nc.vector.memset(out_sb, 0xFFFFFFFF)

        # --- setup: transpose codes and x into [d, token/code] bf16 layout ---
        if stage >= 2:
            with tc.tile_pool(name="tpsum", bufs=4, space="PSUM") as tp:
                for cb in range(CBLK):
                    pt = tp.tile((64, 128), fp32, tag="ct")
                    nc.tensor.transpose(pt, csb[:, cb, :], ident)
                    nc.scalar.copy(cT[0:64, cb * 128:(cb + 1) * 128], pt)
                for xb in range(NBLK):
                    pt = tp.tile((64, 128), fp32, tag="xt")
                    nc.tensor.transpose(pt, xsb[:, xb, :], ident)
                    nc.scalar.copy(xT[0:64, xb * 128:(xb + 1) * 128], pt)

            # cTsq = cT*cT, c2 row: ones.T @ cTsq -> -c2/2 into cT row 64
            nc.vector.tensor_tensor(cTsq, cT[0:64, :], cT[0:64, :], op=Alu.mult)
            with tc.tile_pool(name="c2psum", bufs=2, space="PSUM") as cp:
                for j in range(NCH):
                    pt = cp.tile((1, 512), fp32, tag="c2")
                    nc.tensor.matmul(pt, ones64, cTsq[:, j * 512:(j + 1) * 512],
                                     start=True, stop=True)
                    nc.scalar.activation(cT[64:65, j * 512:(j + 1) * 512], pt,
                                         Act.Copy, bias=0.0, scale=-0.5)

        # x2[t] = sum_d x^2 (fp32): ACT square whole slab, DVE reduce innermost
        if stage >= 3:
            nc.scalar.activation(sq_all, xsb, Act.Square, bias=0.0, scale=1.0)
            nc.vector.tensor_reduce(x2, sq_all, axis=mybir.AxisListType.X,
                                    op=Alu.add)
        else:
            nc.vector.memset(x2, 1.0)

        # --- main loop ---
        if stage >= 4:
            with tc.tile_pool(name="gpsum", bufs=1, space="PSUM") as gp, \
                 tc.tile_pool(name="gsb", bufs=2) as gsb_pool:
                gbanks = [gp.tile((128, 512), fp32, tag=f"g{j}", name=f"g{j}")
                          for j in range(NCH)]
                for blk in range(NBLK):
                    lhsT = xT[:, blk * 128:(blk + 1) * 128]
                    g_sb = gsb_pool.tile((128, M), bf16, tag="g_sb")
                    for j in range(NCH):
                        nc.tensor.matmul(gbanks[j], lhsT,
                                         cT[:, j * 512:(j + 1) * 512],
                                         start=True, stop=True)
                        nc.scalar.copy(g_sb[:, j * 512:(j + 1) * 512], gbanks[j])
                    if stage >= 5:
                        nc.vector.max(top8, g_sb)
                        nc.vector.max_index(idx8, top8, g_sb)
                    if stage >= 6:
                        # argmax id only; the threshold decision happens on
                        # the host with an exact fp64 distance recompute
                        # (bf16 scores are too noisy for borderline cases)
                        nc.vector.tensor_copy(out_sb[:, blk:blk + 1],
                                              idx8[:, 0:1])

        dma.dma_start(out=o_d[:].rearrange("(b p) -> p b", p=128), in_=out_sb)

    nc.compile()
    return nc


KPAD = 128  # contraction rows incl. zero padding: K<96 leaves the PE
            # HAM activity monitor throttled at 1.2 GHz (measured), K>=96
            # runs warm at 2.4 GHz.
VW = 1024   # per-instruction span for the two VectorE consumers
            # (must stay bank-aligned: unaligned spans spread PSUM WAR deps
            # across extra banks and stall the pipeline ~30%)
SW = (4096 - 2 * VW) // 2  # per-instruction span for the two ScalarE ones


def _build_fast2():
    """Screening program: no argmax, no on-device setup.

    Inputs are host-transposed bf16 padded to KPAD contraction rows:
    xt (KPAD, TOK) row 64 = ones, ct (KPAD, M) row 64 = -c2/2, rows
    65+ zero; tau (128, NBLK) = -((x2-0.1)/2 - SOFT).
    Per block: 8 matmuls -> 4 flat 2-bank PSUM tiles; DVE flat
    tensor_reduce(max) tiles 0-1 -> gmax partials (codes 0-2047); ACT
    relu+accum tiles 2-3 -> sum-of-relu partials (codes 2048-4095).
    """
    import concourse.bacc as bacc
    import concourse.mybir as mybir
    import concourse.tile as tile
    from contextlib import ExitStack

    fp32 = mybir.dt.float32
    bf16 = mybir.dt.bfloat16
    Alu = mybir.AluOpType
    Act = mybir.ActivationFunctionType

    nc = bacc.Bacc(
        "TRN2",
        target_bir_lowering=False,
        debug=False,
        enable_asserts=False,
        num_devices=1,
    )

    xt_d = nc.dram_tensor("xt", (KPAD, TOK), bf16, kind="ExternalInput")
    ct_d = nc.dram_tensor("ct", (KPAD, M), bf16, kind="ExternalInput")
    tau_d = nc.dram_tensor("tau", (128, NBLK), fp32, kind="ExternalInput")
    outp_d = nc.dram_tensor("outp", (128, NBLK, 4), fp32, kind="ExternalOutput")

    with tile.TileContext(nc) as tc, ExitStack() as ctx:
        sb = ctx.enter_context(tc.tile_pool(name="sb", bufs=1))

        xt_sb = sb.tile((KPAD, TOK), bf16, tag="xt_sb")
        ct_sb = sb.tile((KPAD, M), bf16, tag="ct_sb")
        tau_sb = sb.tile((128, NBLK), fp32, tag="tau_sb")
        outp_sb = sb.tile((128, NBLK, 4), fp32, tag="outp_sb")
        gacc = sb.tile((128, SW), bf16, tag="gacc")
        warm = sb.tile((128, 1), fp32, tag="warm")
        dmy = sb.tile((128, 512), bf16, tag="dmy")

        dma = nc.default_dma_engine
        # ACT table warm-up: pay the one-time table load under the DMA wait;
        # GPSIMD warm-up: trigger its ucode load early; zero accumulators.
        nc.vector.memset(warm, 0.0)
        nc.scalar.activation(warm, warm, Act.Relu, bias=0.0, scale=1.0)
        nc.vector.memset(outp_sb, 0.0)
        nc.vector.memset(dmy, 0.0)
        nc.gpsimd.memset(gacc, 0.0)
        nc.gpsimd.tensor_tensor(gacc[:, 0:8], gacc[:, 0:8], gacc[:, 0:8],
                                op=Alu.add)

        # DMA order: first block's operands first, finely chunked so early
        # matmuls are not gated on one large transfer; then the rest in a
        # few large transfers (each dma_start costs ~0.7us of issue time).
        dma.dma_start(out=ct_sb[:, 0:512], in_=ct_d[:, 0:512])
        dma.dma_start(out=xt_sb[:, 0:128], in_=xt_d[:, 0:128])
        dma.dma_start(out=tau_sb, in_=tau_d[:, :])
        dma.dma_start(out=ct_sb[:, 512:1024], in_=ct_d[:, 512:1024])
        dma.dma_start(out=ct_sb[:, 1024:2048], in_=ct_d[:, 1024:2048])
        dma.dma_start(out=xt_sb[:, 128:1024], in_=xt_d[:, 128:1024])
        dma.dma_start(out=ct_sb[:, 2048:4096], in_=ct_d[:, 2048:4096])
        dma.dma_start(out=xt_sb[:, 1024:4096], in_=xt_d[:, 1024:4096])
        dma.dma_start(out=xt_sb[:, 4096:TOK], in_=xt_d[:, 4096:TOK])

        with tc.tile_pool(name="gpsum", bufs=1, space="PSUM") as gp, \
             tc.tile_pool(name="scr", bufs=2) as scr_pool:
            pall = gp.tile((128, 4096), fp32, tag="pall", name="pall")
            # dummy matmuls during the DMA wait: build up the PE activity
            # window so the HAM clock un-throttles before real work arrives
            for w in range(5):
                nc.tensor.matmul(pall[:, 3584:4096], dmy[:, 0:128], dmy,
                                 start=True, stop=True)
            for blk in range(NBLK):
                lhsT = xt_sb[:, blk * 128:(blk + 1) * 128]
                for j in range(NCH):
                    nc.tensor.matmul(pall[:, j * 512:(j + 1) * 512], lhsT,
                                     ct_sb[:, j * 512:(j + 1) * 512],
                                     start=True, stop=True)
                # DVE: relu(g - tau) + accum over the first ~3.8 banks;
                # ScalarE (faster per element) takes the remainder
                for h in range(2):
                    lo, hi = h * VW, (h + 1) * VW
                    vscr = scr_pool.tile((128, VW), bf16, tag=f"v{h}")
                    nc.vector.tensor_scalar(out=vscr, in0=pall[:, lo:hi],
                                            scalar1=tau_sb[:, blk:blk + 1],
                                            scalar2=0.0,
                                            op0=Alu.add, op1=Alu.max,
                                            accum_out=outp_sb[:, blk,
                                                              h:h + 1])
                # ACT: relu(g - tau) over banks 4-5, strip folded by GPSIMD
                # (saves one accumulator-read per block on ScalarE)
                # the last block's strip accumulates directly so the gacc
                # fold (strips 0-62) runs one block early, off the tail
                last = blk == NBLK - 1
                if last:
                    gscr = scr_pool.tile((128, SW), bf16, tag="gscr")
                    nc.vector.tensor_scalar(out=gscr, in0=gacc,
                                            scalar1=0.0, scalar2=0.0,
                                            op0=Alu.add, op1=Alu.max,
                                            accum_out=outp_sb[:, blk - 1,
                                                              3:4])
                s1 = scr_pool.tile((128, SW), bf16, tag="s1")
                nc.scalar.activation(s1, pall[:, 2 * VW:2 * VW + SW], Act.Relu,
                                     bias=tau_sb[:, blk:blk + 1], scale=1.0,
                                     accum_out=(outp_sb[:, blk, 3:4]
                                                if last else None))
                if not last:
                    nc.gpsimd.tensor_tensor(gacc, gacc, s1, op=Alu.add)
                # ACT: relu(g - tau) + accum over banks 6-7
                s2 = scr_pool.tile((128, SW), bf16, tag="s2")
                nc.scalar.activation(s2, pall[:, 2 * VW + SW:4096], Act.Relu,
                                     bias=tau_sb[:, blk:blk + 1], scale=1.0,
                                     accum_out=outp_sb[:, blk, 2:3])

        dma.dma_start(out=outp_d[:, :, :], in_=outp_sb)

    nc.compile()
    return nc


def _run(nc, in_maps, trace):
    from concourse import bass_utils
    try:
        return bass_utils.run_bass_kernel_spmd(
            nc, in_maps, list(range(NCORES)), trace=trace)
    except Exception:
        if not trace:
            raise
        return bass_utils.run_bass_kernel_spmd(
            nc, in_maps, list(range(NCORES)), trace=False)


def _full_in_maps(x, codes):
    ident = np.eye(128, dtype=np.float32)
    xf = x.reshape(NCORES, TOK, D)
    return [
        {"x": xf[c], "codes": codes, "ident": ident}
        for c in range(NCORES)
    ]


def _run_full(x, codes, trace):
    if "full" not in _CACHE:
        _CACHE["full"] = _build(6)
    res = _run(_CACHE["full"], _full_in_maps(x, codes), trace)
    _CACHE["last_res"] = res
    ids = np.concatenate(
        [np.asarray(res.results[c]["out"], dtype=np.uint32)
         for c in range(NCORES)]).astype(np.int64)
    # exact threshold decision on the device-picked nearest candidates
    xf = x.reshape(-1, D).astype(np.float64)
    d2 = ((xf - codes.astype(np.float64)[ids]) ** 2).sum(axis=1)
    out = np.where(d2 <= THRESH, ids, -1).astype(np.int32)
    return out.reshape(B, N)


def kernel(x: np.ndarray, codes: np.ndarray) -> np.ndarray:
    import ml_dtypes

    os.environ.setdefault("NEURON_RT_RESET_CORES", "1")
    x = np.ascontiguousarray(x, dtype=np.float32)
    codes = np.ascontiguousarray(codes, dtype=np.float32)
    trace = bool(os.environ.get("KERNEL_TRACE"))

    if os.environ.get("KERNEL_FORCE_FULL"):
        return _run_full(x, codes, trace)

    bf16 = ml_dtypes.bfloat16
    xf = x.reshape(NCORES, TOK, D)

    ct = np.zeros((KPAD, M), dtype=bf16)
    ct[0:64] = codes.T.astype(bf16)
    c2 = (codes.astype(np.float64) ** 2).sum(axis=1)
    ct[64] = (-0.5 * c2).astype(bf16)

    in_maps = []
    for c in range(NCORES):
        slab = xf[c]
        xt = np.zeros((KPAD, TOK), dtype=bf16)
        xt[0:64] = slab.T.astype(bf16)
        xt[64] = np.float32(1.0)
        x2 = (slab.astype(np.float64) ** 2).sum(axis=1)          # (TOK,)
        x2_pb = x2.reshape(NBLK, 128).T.astype(np.float32)       # (128, NBLK)
        tau = np.ascontiguousarray(
            -((x2_pb - THRESH) * 0.5 - SOFT).astype(np.float32))
        in_maps.append({"xt": xt, "ct": np.ascontiguousarray(ct), "tau": tau})

    if "fast2" not in _CACHE:
        _CACHE["fast2"] = _build_fast2()
    res = _run(_CACHE["fast2"], in_maps, trace)
    _CACHE["last_res"] = res

    ok = True
    for c in range(NCORES):
        outp = np.asarray(res.results[c]["outp"], dtype=np.float32)
        # sum-of-relu certificate: exactly 0 iff every code is far from
        # every token (any true match contributes >= SOFT - bf16 error)
        if outp.max() > 0.05:
            ok = False
            break
    if ok:
        return np.full((B, N), -1, dtype=np.int32)

    return _run_full(x, codes, trace)
